# revision 6
# baseline (speedup 1.0000x reference)
"""Trainium2 Bass kernel for nn_BoundaryLoss_49306224558104.

Math note: in the reference, every pixel is either foreground (where
neg = edt(~fg) is exactly 0) or background (where pos = edt(fg) is
exactly 0), so min(pos, neg) == 0 at every pixel and dist_map is
identically zero (bitwise-exact in f32: the EDT of a pixel whose own
d0 is 0 takes the y==j / k==i branch with cost 0, and sqrt(0) == 0).
The loss therefore reduces exactly to mean(softplus(x) - x*z) with
x = pred.squeeze(1), z = (target > 0).

Sharding: pure data-parallel — sample b goes to core b (B == 8 ==
n_cores). Each core computes per-partition partial sums of
softplus(x) - x*z over its [256,256] sample laid out as a [128,512]
SBUF tile; the host sums the 8x[128] partials and divides by B*H*W.
"""

import numpy as np

B, H, W = 8, 256, 256
P, F = 128, 512  # H*W == P*F
N_CORES = 8


def _build_nc():
    import concourse.bass as bass
    import concourse.mybir as mybir

    nc = bass.Bass(trn_type="TRN2")

    pred = nc.declare_dram_parameter("pred", [P, F], mybir.dt.float32, isOutput=False)
    targ = nc.declare_dram_parameter("target", [P, F], mybir.dt.int32, isOutput=False)
    out = nc.declare_dram_parameter("out", [P, 1], mybir.dt.float32, isOutput=True)

    with (
        nc.sbuf_tensor("x", [P, F], mybir.dt.float32) as x,
        nc.sbuf_tensor("t", [P, F], mybir.dt.int32) as t,
        nc.sbuf_tensor("e", [P, F], mybir.dt.float32) as e,
        nc.sbuf_tensor("l", [P, F], mybir.dt.float32) as l,
        nc.sbuf_tensor("xz", [P, F], mybir.dt.float32) as xz,
        nc.sbuf_tensor("sp_sum", [P, 1], mybir.dt.float32) as sp_sum,
        nc.sbuf_tensor("xz_sum", [P, 1], mybir.dt.float32) as xz_sum,
        nc.sbuf_tensor("partial", [P, 1], mybir.dt.float32) as partial,
        nc.semaphore("x_sem") as x_sem,
        nc.semaphore("t_sem") as t_sem,
        nc.semaphore("s_sem") as s_sem,
        nc.semaphore("a_sem") as a_sem,
        nc.semaphore("v_sem") as v_sem,
        nc.semaphore("g_sem") as g_sem,
        nc.semaphore("o_sem") as o_sem,
        nc.Block() as block,
    ):

        @block.sync
        def _(sync):
            sync.dma_start(out=x[:, :], in_=pred[:, :]).then_inc(x_sem, 16)
            sync.dma_start(out=t[:, :], in_=targ[:, :]).then_inc(t_sem, 16)
            sync.wait_ge(g_sem, 1)
            sync.dma_start(out=out[:, :], in_=partial[:, :]).then_inc(o_sem, 16)
            sync.wait_ge(o_sem, 16)

        @block.scalar
        def _(scalar):
            # softplus(x) = ln(1 + exp(x)); inputs are N(0,1) logits, |x| << 88
            # so the direct form neither overflows nor loses precision.
            scalar.wait_ge(x_sem, 16)
            scalar.activation(e[:, :], x[:, :], mybir.ActivationFunctionType.Exp)
            # same-engine RAW on `e`: flush the ACT pipeline before Ln reads it
            # (a bare drain() fails walrus codegen; give it a sem update)
            scalar.drain().then_inc(s_sem, 1)
            scalar.wait_ge(s_sem, 1)
            scalar.activation(
                l[:, :],
                e[:, :],
                mybir.ActivationFunctionType.Ln,
                bias=1.0,
                accum_out=sp_sum[:, :],
            ).then_inc(a_sem, 1)

        @block.vector
        def _(vector):
            vector.wait_ge(x_sem, 16)
            vector.wait_ge(t_sem, 16)
            # xz = (x * 1.0) * t (int32 operand converted on read);
            # xz_sum = row-sum. (tensor_tensor_reduce is broken in this
            # walrus build — "ISA wrong length" — scalar_tensor_tensor with
            # accum_out is the working equivalent.)
            vector.scalar_tensor_tensor(
                out=xz[:, :],
                in0=x[:, :],
                scalar=1.0,
                in1=t[:, :],
                op0=mybir.AluOpType.mult,
                op1=mybir.AluOpType.mult,
                accum_out=xz_sum[:, :],
            ).then_inc(v_sem, 1)

        @block.gpsimd
        def _(gpsimd):
            gpsimd.wait_ge(a_sem, 1)
            gpsimd.wait_ge(v_sem, 1)
            gpsimd.tensor_sub(partial[:, :], sp_sum[:, :], xz_sum[:, :]).then_inc(
                g_sem, 1
            )

    return nc


def kernel(pred: np.ndarray, target: np.ndarray) -> np.ndarray:
    from concourse.bass_utils import run_bass_kernel_spmd

    x = np.ascontiguousarray(pred.reshape(B, P, F)).astype(np.float32, copy=False)
    t = np.ascontiguousarray(target.reshape(B, P, F)).astype(np.int32, copy=False)

    nc = _build_nc()
    in_maps = [{"pred": x[b], "target": t[b]} for b in range(B)]
    res = run_bass_kernel_spmd(nc, in_maps, list(range(N_CORES)))

    total = 0.0
    for r in res.results:
        total += float(r["out"].astype(np.float64).sum())
    return np.array(total / (B * H * W), dtype=np.float32)


# revision 17
# speedup vs baseline: 1.4356x; 1.4356x over previous
"""Trainium2 Bass kernel for nn_BoundaryLoss_49306224558104.

Math note: in the reference, every pixel is either foreground (where
neg = edt(~fg) is exactly 0) or background (where pos = edt(fg) is
exactly 0), so min(pos, neg) == 0 at every pixel and dist_map is
identically zero (bitwise-exact in f32: the EDT of a pixel whose own
d0 is 0 takes the y==j / k==i branch with cost 0, and sqrt(0) == 0).
The loss therefore reduces exactly to mean(softplus(x) - x*z) with
x = pred.squeeze(1), z = (target > 0).

Sharding: pure data-parallel — sample b goes to core b (B == 8 ==
n_cores). Per core, the sample's pred (f32) and target (cast to f32
on host) are packed into one [128, 1024] DRAM buffer; the two halves
are DMA'd on the two HWDGE rings (sync + scalar engines) in parallel.
softplus(x) = ln(1 + exp(x)) on the scalar engine (inputs are N(0,1)
logits, |x| << 88, so the direct form neither overflows nor loses
precision; this build's act tables have exp+ln in one set but no
softplus). Row sums come from the activation / scalar_tensor_tensor
accumulators; a ones-vector matmul on the tensor engine collapses the
128 partition partials to a single [1, 2] PSUM value so the output
DMA is one 8-byte descriptor (a [128, 1] per-partition DMA costs
~7 us in descriptor processing). Host combines 8 x [1, 2] partials
into the scalar mean.
"""

import numpy as np

B, H, W = 8, 256, 256
P, F = 128, 512  # H*W == P*F
FX2 = 2 * F
N_CORES = 8


def _build_nc():
    import concourse.bass as bass
    import concourse.mybir as mybir

    nc = bass.Bass(trn_type="TRN2")

    xt = nc.declare_dram_parameter("xt", [P, FX2], mybir.dt.float32, isOutput=False)
    out = nc.declare_dram_parameter("out", [1, 2], mybir.dt.float32, isOutput=True)

    zeros128 = nc.const_aps.aps[(mybir.dt.float32, 0.0)]  # [128,1] framework const
    ones128 = nc.const_aps.aps[(mybir.dt.float32, 1.0)]  # [128,1] framework const

    with (
        nc.sbuf_tensor("xtt", [P, FX2], mybir.dt.float32) as xtt,
        nc.sbuf_tensor("e", [P, F], mybir.dt.float32) as e,
        nc.sbuf_tensor("l", [P, F], mybir.dt.float32) as l,
        nc.sbuf_tensor("xz", [P, F], mybir.dt.float32) as xz,
        nc.sbuf_tensor("sums", [P, 2], mybir.dt.float32) as sums,
        nc.sbuf_tensor("trash", [P, 1], mybir.dt.float32) as trash,
        nc.sbuf_tensor("res", [1, 2], mybir.dt.float32) as res,
        nc.psum_tensor("ps", [1, 2], mybir.dt.float32) as ps,
        nc.psum_tensor("ps_warm", [1, 2], mybir.dt.float32) as ps_warm,
        nc.semaphore("x_sem") as x_sem,
        nc.semaphore("t_sem") as t_sem,
        nc.semaphore("s_sem") as s_sem,
        nc.semaphore("a_sem") as a_sem,
        nc.semaphore("v_sem") as v_sem,
        nc.semaphore("m_sem") as m_sem,
        nc.semaphore("r_sem") as r_sem,
        nc.semaphore("o_sem") as o_sem,
        nc.Block() as block,
    ):
        x = xtt[:, 0:F]  # pred logits
        tf = xtt[:, F:FX2]  # target as f32

        @block.sync
        def _(sync):
            sync.dma_start(out=xtt[:, 0:F], in_=xt[:, 0:F]).then_inc(x_sem, 16)
            sync.wait_ge(r_sem, 1)
            sync.dma_start(out=out[:, :], in_=res[:, :]).then_inc(o_sem, 16)
            sync.wait_ge(o_sem, 16)

        @block.scalar
        def _(scalar):
            # second input half on the ACT HWDGE ring, parallel to sync's
            scalar.dma_start(out=xtt[:, F:FX2], in_=xt[:, F:FX2]).then_inc(t_sem, 16)
            # dummy activation: forces the exp/ln PWP table load to happen
            # here, hidden under the input DMA, not on the critical path
            scalar.activation(trash[:, :], zeros128, mybir.ActivationFunctionType.Exp)
            scalar.wait_ge(x_sem, 16)
            scalar.activation(e[:, :], x, mybir.ActivationFunctionType.Exp)
            # same-engine RAW on `e`: flush the ACT pipeline before Ln reads
            # it (a bare drain() fails walrus codegen; give it a sem update)
            scalar.drain().then_inc(s_sem, 1)
            scalar.wait_ge(s_sem, 1)
            scalar.activation(
                l[:, :],
                e[:, :],
                mybir.ActivationFunctionType.Ln,
                bias=1.0,
                accum_out=sums[:, 0:1],
            ).then_inc(a_sem, 1)

        @block.vector
        def _(vector):
            vector.wait_ge(x_sem, 16)
            vector.wait_ge(t_sem, 16)
            # xz = (x * 1.0) * tf ; sums[:,1] = row-sum(xz)
            # (tensor_tensor_reduce is broken in this walrus build — "ISA
            # wrong length" — scalar_tensor_tensor+accum is the equivalent.)
            vector.scalar_tensor_tensor(
                out=xz[:, :],
                in0=x,
                scalar=1.0,
                in1=tf,
                op0=mybir.AluOpType.mult,
                op1=mybir.AluOpType.mult,
                accum_out=sums[:, 1:2],
            ).then_inc(v_sem, 1)
            # bounce the matmul result PSUM -> SBUF (DMA can't read PSUM)
            vector.wait_ge(m_sem, 1)
            vector.tensor_copy(res[:, :], ps[:, :]).then_inc(r_sem, 1)

        @block.tensor
        def _(tensor):
            # warm-up matmul: wakes the PE and pipelines the ldweights path
            # while the input DMA is still in flight (reads a never-written
            # tile; result is discarded)
            tensor.matmul(ps_warm[:, 0:1], ones128, ones128, start=True, stop=True)
            tensor.wait_ge(a_sem, 1)
            tensor.wait_ge(v_sem, 1)
            # [1,2] = ones[128,1].T @ sums[128,2] — collapses partitions
            tensor.matmul(
                ps[:, :], ones128, sums[:, :], start=True, stop=True
            ).then_inc(m_sem, 1)

    return nc


def kernel(pred: np.ndarray, target: np.ndarray) -> np.ndarray:
    from concourse.bass_utils import run_bass_kernel_spmd

    xt = np.empty((B, P, FX2), dtype=np.float32)
    xt[:, :, :F] = pred.reshape(B, P, F)
    xt[:, :, F:] = target.reshape(B, P, F).astype(np.float32)

    nc = _build_nc()
    in_maps = [{"xt": xt[b]} for b in range(B)]
    res = run_bass_kernel_spmd(nc, in_maps, list(range(N_CORES)))

    total = 0.0
    for r in res.results:
        o = r["out"].astype(np.float64)
        total += o[0, 0] - o[0, 1]
    return np.array(total / (B * H * W), dtype=np.float32)
